# revision 31
# baseline (speedup 1.0000x reference)
"""DecoderRNN (show-attend-tell style) on 8 trn2 NeuronCores.

Strategy: data-parallel over batch (8 batches/core, weights replicated,
no collectives).  The per-step context matmul is folded into a one-time
precompute proj = features @ W_ih[:, E:].T so the per-step attention
weighted sum runs in gate space as a block-diagonal matmul on the PE.
The vocab projection runs after the scan as one batched matmul over all
(step, batch) hidden states.
"""
import numpy as np
import ml_dtypes

# ---- problem constants (hardcoded; kernel.py must be self-contained) ----
B, N, ENC = 64, 49, 2048
S = 26
V, E, H, A = 10000, 512, 512, 512
G = 4 * H            # 2048 gate dim
T = S - 1            # 25 decode steps
NC_ = 8              # cores
BL = B // NC_        # 8 batches per core
NP = 64              # n padded to 64 -> (b, n) pack = 512 rows
BN = BL * NP         # 512
VC = 500             # vocab chunk for phase 2
BF = ml_dtypes.bfloat16

TRACE = False        # test.py can flip this for profiled runs
LAST_RESULTS = None  # BassKernelResults stash for test.py


# ---------------------------------------------------------------- wait split
def _split_multi_waits_json(bir: dict) -> dict:
    """walrus in this env supports 1 sync-wait per instruction; split the
    extra semaphore waits Tile emits into standalone EventSemaphore insts."""
    n = 0
    for f in bir.get("functions", []):
        for bb in f.get("blocks", []):
            out = []
            for inst in bb.get("instructions", []):
                si = inst.get("sync_info")
                waits = (si or {}).get("on_wait") or []
                if len(waits) > 1:
                    sem = [w for w in waits if w.get("sync_type") == "semaphore"]
                    other = [w for w in waits if w.get("sync_type") != "semaphore"]
                    keep, move = (other, sem) if other else ([sem[-1]], sem[:-1])
                    for w in move:
                        n += 1
                        out.append({
                            "debug": inst.get("debug", 0),
                            "engine": inst["engine"],
                            "ins": [], "outs": [],
                            "name": f"{inst['name']}-ws{n}",
                            "opcode": "EventSemaphore",
                            "sync_info": {"on_update": [], "on_wait": [w]},
                        })
                    si["on_wait"] = keep
                out.append(inst)
            bb["instructions"] = out
    return bir


_PATCHED = False


def _install_wait_split():
    global _PATCHED
    if _PATCHED:
        return
    _PATCHED = True
    import orjson
    import concourse.bass as bass
    orig = bass.Bass.to_json_bytes

    def patched(self) -> bytes:
        return orjson.dumps(_split_multi_waits_json(orjson.loads(orig(self))))

    bass.Bass.to_json_bytes = patched


# ---------------------------------------------------------------- program
def build_program():
    import concourse.bass as bass
    import concourse.mybir as mybir
    from concourse.tile import TileContext

    BF16 = mybir.dt.bfloat16
    F32 = mybir.dt.float32
    AF = mybir.ActivationFunctionType
    OP = mybir.AluOpType
    AX = mybir.AxisListType

    nc = bass.Bass()
    dp = lambda name, shape, dt=BF16: nc.declare_dram_parameter(name, shape, dt, isOutput=False)

    # per-core inputs (already transposed/padded/bf16 on host)
    featT_d = dp("featT", [128, 16 * BN])          # [e_c, 128][bn]
    embT_d = dp("embT", [128, 4 * T * BL])         # [e_c, 128][s*8+b]
    wdec_d = dp("wdec", [128, 4 * A])              # lhsT[k=h][a], chunk h_c at a-col block
    wihe_d = dp("wihe", [128, 4 * G])              # rhs [k=e][g]
    wihc_d = dp("wihc", [128, 16 * G])             # rhs [k=enc][g]
    whh_d = dp("whh", [128, 4 * G])                # rhs [k=h][g]
    wenc_d = dp("wenc", [128, 16 * A])             # lhsT [k=enc][a]
    wfull_d = dp("wfull", [128, 4])                # lhsT [k=a][1] per chunk col
    wfcn_d = dp("wfcn", [128, 4 * V])              # rhs [k=h][v]
    winh_d = dp("winh", [128, 16 * H])             # rhs [k=enc][h]
    winc_d = dp("winc", [128, 16 * H])             # rhs [k=enc][h]
    batt_d = dp("batt", [1, A])                    # b_enc_att + b_dec_att
    bg_d = dp("bg", [1, G])                        # b_ih + b_hh
    binh_d = dp("binh", [1, H])
    binc_d = dp("binc", [1, H])
    bfcn_d = dp("bfcn", [1, V])
    ones_d = dp("ones", [1, 128])
    onesbn_d = dp("onesbn", [1, BN])
    id8_d = dp("id8", [8, 8])
    id1_d = dp("id1", [1, 1], F32)
    maskbn_d = dp("maskbn", [1, 128])

    preds_d = nc.declare_dram_parameter("preds", [BL, T, V], F32, isOutput=True)
    alphas_d = nc.declare_dram_parameter("alphas", [BL, T, N], F32, isOutput=True)

    with TileContext(nc) as tc:
        with tc.tile_pool(name="main", bufs=1) as mp:
            # ---- persistent SBUF ----
            # tiles for scan weights (DMAs issued after precompute-critical
            # loads so they don't delay featT/wihc on the DMA queues)
            wihe = mp.tile([128, 4 * G], BF16)
            whh = mp.tile([128, 4 * G], BF16)
            wdec = mp.tile([128, 4 * A], BF16)
            wfull = mp.tile([128, 4], BF16)
            embT = mp.tile([128, 4 * T * BL], BF16)
            bg = mp.tile([1, G], BF16)
            nc.sync.dma_start(out=bg[:], in_=bg_d[:])
            bfcn = mp.tile([1, V], BF16)
            ones = mp.tile([1, 128], BF16)
            nc.sync.dma_start(out=ones[:], in_=ones_d[:])
            id8 = mp.tile([8, 8], BF16)
            id1 = mp.tile([1, 1], F32)
            maskbn = mp.tile([1, 128], BF16)
            nc.sync.dma_start(out=maskbn[:], in_=maskbn_d[:])

            proj = mp.tile([128, 4 * G], BF16)        # [(b,n) chunk][g]
            att1 = mp.tile([128, 4 * BN], BF16)       # [a chunk][bn]
            h_store = mp.tile([128, S * 32], BF16)    # col h_c*208 + s*8 + b
            alpha_diag = mp.tile([128, 32], BF16)     # col bn_c*8 + b
            nc.vector.memset(alpha_diag[:], 0.0)
            c_buf = [mp.tile([8, H], F32, name=f"cbuf{i}") for i in range(2)]

            # =================== PRECOMPUTE ===================
            with (
                tc.tile_pool(name="pre", bufs=1) as pp,
                tc.tile_pool(name="pre_dma", bufs=2) as pd,
                tc.tile_pool(name="pre_ps", bufs=1, space="PSUM") as pps,
            ):
                featT = pp.tile([128, 16 * BN], BF16)
                for c in range(8):
                    w = 2 * BN
                    nc.sync.dma_start(out=featT[:, c * w:(c + 1) * w],
                                      in_=featT_d[:, c * w:(c + 1) * w])
                wenc = pp.tile([128, 16 * A], BF16)
                for c in range(4):
                    w = 4 * A
                    nc.sync.dma_start(out=wenc[:, c * w:(c + 1) * w],
                                      in_=wenc_d[:, c * w:(c + 1) * w])
                batt = pp.tile([1, A], BF16)
                nc.sync.dma_start(out=batt[:], in_=batt_d[:])
                binh = pp.tile([1, H], BF16)
                nc.sync.dma_start(out=binh[:], in_=binh_d[:])
                binc = pp.tile([1, H], BF16)
                nc.sync.dma_start(out=binc[:], in_=binc_d[:])
                onesbn = pp.tile([1, BN], BF16)
                nc.sync.dma_start(out=onesbn[:], in_=onesbn_d[:])
                winh_s = pp.tile([128, 16 * H], BF16)
                winc_s = pp.tile([128, 16 * H], BF16)
                for c in range(4):
                    w = 4 * H
                    nc.sync.dma_start(out=winh_s[:, c * w:(c + 1) * w],
                                      in_=winh_d[:, c * w:(c + 1) * w])
                    nc.sync.dma_start(out=winc_s[:, c * w:(c + 1) * w],
                                      in_=winc_d[:, c * w:(c + 1) * w])

                # mean_enc (pad cols are zero so plain sum/49 is exact)
                meanf = pp.tile([128, 16 * BL], F32)
                for e_c in range(16):
                    nc.vector.tensor_reduce(
                        meanf[:, e_c * BL:(e_c + 1) * BL],
                        featT[:, e_c * BN:(e_c + 1) * BN].rearrange(
                            "p (b n) -> p b n", n=NP),
                        axis=AX.X, op=OP.add)
                mean = pp.tile([128, 16 * BL], BF16)
                nc.vector.tensor_scalar_mul(mean[:], meanf[:], 1.0 / N)

                # proj = features @ W_ihc.T  -> [(b,n), g]
                for g_c in range(4):
                    wslab = pd.tile([128, 16 * 512], BF16, name="wslab")
                    nc.sync.dma_start(
                        out=wslab[:],
                        in_=wihc_d[:].rearrange("p (e g) -> p e g", g=G)[
                            :, :, g_c * 512:(g_c + 1) * 512])
                    for bn_c in range(4):
                        ps = pps.tile([128, 512], F32, name="proj_ps", bufs=2)
                        for e_c in range(16):
                            nc.tensor.matmul(
                                ps[:],
                                featT[:, e_c * BN + bn_c * 128: e_c * BN + (bn_c + 1) * 128],
                                wslab[:, e_c * 512:(e_c + 1) * 512],
                                start=(e_c == 0), stop=False)
                        nc.tensor.matmul(
                            ps[:], maskbn[:], bg[:, g_c * 512:(g_c + 1) * 512],
                            start=False, stop=True)
                        nc.vector.tensor_copy(
                            proj[:, bn_c * G + g_c * 512: bn_c * G + (g_c + 1) * 512],
                            ps[:])

                # att1 = features @ W_enc_att.T + (b_enc+b_dec)  -> [a, (b,n)]
                for a_c in range(4):
                    ps = pps.tile([128, BN], F32, name="att1_ps")
                    for e_c in range(16):
                        nc.tensor.matmul(
                            ps[:],
                            wenc[:, e_c * A + a_c * 128: e_c * A + (a_c + 1) * 128],
                            featT[:, e_c * BN:(e_c + 1) * BN],
                            start=(e_c == 0), stop=False)
                    nc.tensor.matmul(
                        ps[:], batt[:, a_c * 128:(a_c + 1) * 128], onesbn[:],
                        start=False, stop=True)
                    nc.vector.tensor_copy(
                        att1[:, a_c * BN:(a_c + 1) * BN], ps[:])

                # scan weights can stream in while proj/att1 matmuls run
                nc.sync.dma_start(out=wdec[:], in_=wdec_d[:])
                nc.sync.dma_start(out=wfull[:], in_=wfull_d[:])
                nc.sync.dma_start(out=wihe[:], in_=wihe_d[:])
                nc.sync.dma_start(out=whh[:], in_=whh_d[:])
                nc.sync.dma_start(out=embT[:], in_=embT_d[:])
                nc.sync.dma_start(out=id8[:], in_=id8_d[:])
                nc.sync.dma_start(out=id1[:], in_=id1_d[:])
                nc.sync.dma_start(out=bfcn[:], in_=bfcn_d[:])

                # h0 / c0
                h0ps = pps.tile([8, H], F32, name="h0_ps")
                c0ps = pps.tile([8, H], F32, name="c0_ps")
                for e_c in range(16):
                    nc.tensor.matmul(h0ps[:], mean[:, e_c * BL:(e_c + 1) * BL],
                                     winh_s[:, e_c * H:(e_c + 1) * H],
                                     start=(e_c == 0), stop=False)
                    nc.tensor.matmul(c0ps[:], mean[:, e_c * BL:(e_c + 1) * BL],
                                     winc_s[:, e_c * H:(e_c + 1) * H],
                                     start=(e_c == 0), stop=False)
                nc.tensor.matmul(h0ps[:], ones[:, :BL], binh[:], start=False, stop=True)
                nc.tensor.matmul(c0ps[:], ones[:, :BL], binc[:], start=False, stop=True)
                nc.vector.tensor_copy(c_buf[0][:], c0ps[:])
                h0 = pp.tile([8, H], BF16)
                nc.vector.tensor_copy(h0[:], h0ps[:])
                hTps = pps.tile([128, 32], F32, name="hT_ps0")
                for h_c in range(4):
                    nc.tensor.matmul(hTps[:, h_c * 8:(h_c + 1) * 8],
                                     h0[:, h_c * 128:(h_c + 1) * 128], id8[:],
                                     start=True, stop=True)
                for h_c in range(4):
                    nc.vector.tensor_copy(
                        h_store[:, h_c * 208: h_c * 208 + 8],
                        hTps[:, h_c * 8:(h_c + 1) * 8])

            # =================== SCAN + VOCAB ===================
            with (
                tc.tile_pool(name="fcnp", bufs=1) as fp,
                tc.tile_pool(name="step", bufs=1) as sp,
            ):
                wfcn = fp.tile([128, 4 * V], BF16)
                for c in range(8):
                    w = V // 2
                    nc.sync.dma_start(out=wfcn[:, c * w:(c + 1) * w],
                                      in_=wfcn_d[:, c * w:(c + 1) * w])

                with tc.tile_pool(name="scan_ps", bufs=1, space="PSUM") as ps_pool:
                    def emit_pred_half(s0, ns):
                        # preds[b, s, :] = h_{s+1} @ W_fcn.T + b_fcn
                        m = ns * 8
                        for v_c in range(V // VC):
                            ps = ps_pool.tile([104, VC], F32, name="pred_ps",
                                              bufs=2)
                            for h_c in range(4):
                                nc.tensor.matmul(
                                    ps[:m],
                                    h_store[:, h_c * 208 + (1 + s0) * 8:
                                            h_c * 208 + (1 + s0) * 8 + m],
                                    wfcn[:, h_c * V + v_c * VC: h_c * V + (v_c + 1) * VC],
                                    start=(h_c == 0), stop=False)
                            nc.tensor.matmul(
                                ps[:m], ones[:, :m], bfcn[:, v_c * VC:(v_c + 1) * VC],
                                start=False, stop=True)
                            pout = sp.tile([104, VC], F32, name="pout", bufs=3)
                            nc.vector.tensor_copy(pout[:m], ps[:m])
                            nc.sync.dma_start(
                                out=preds_d[:].rearrange("b s v -> s b v")[
                                    s0:s0 + ns, :, v_c * VC:(v_c + 1) * VC],
                                in_=pout[:m])

                    for s in range(T):
                        hof = s * 8
                        # att2_T[a, b] = W_dec_att @ h
                        att2 = ps_pool.tile([128, 32], F32, name="s1_ps")
                        for a_c in range(4):
                            for h_c in range(4):
                                nc.tensor.matmul(
                                    att2[:, a_c * 8:(a_c + 1) * 8],
                                    wdec[:, h_c * A + a_c * 128: h_c * A + (a_c + 1) * 128],
                                    h_store[:, h_c * 208 + hof: h_c * 208 + hof + 8],
                                    start=(h_c == 0), stop=(h_c == 3))
                        # R = relu(att1 + att2 bcast over n)  (bf16, PE rhs)
                        att2s = sp.tile([128, 32], BF16, name="att2s")
                        nc.vector.tensor_copy(att2s[:], att2[:])
                        R = sp.tile([128, 4 * BN], BF16, name="R", bufs=2)
                        nc.vector.tensor_tensor(
                            R[:].rearrange("p (c b n) -> p c b n", b=8, n=NP),
                            att1[:].rearrange("p (c b n) -> p c b n", b=8, n=NP),
                            att2s[:].rearrange("p (c b) -> p c b", b=8)
                                .unsqueeze(3).broadcast_to([128, 4, 8, NP]),
                            op=OP.add)
                        nc.vector.tensor_scalar_max(R[:], R[:], 0.0)
                        # gates accumulation starts with the emb part -- emitted
                        # here so the PE chews on it while DVE runs relu/softmax
                        gps = [ps_pool.tile([8, 512], F32, name=f"g{g_c}_ps")[:]
                               for g_c in range(4)]
                        for g_c in range(4):
                            for e_c in range(4):
                                nc.tensor.matmul(
                                    gps[g_c],
                                    embT[:, e_c * T * BL + s * BL: e_c * T * BL + (s + 1) * BL],
                                    wihe[:, e_c * G + g_c * 512: e_c * G + (g_c + 1) * 512],
                                    start=(e_c == 0), stop=False)
                        # e[(b,n)] = W_full . R
                        e_ps = ps_pool.tile([1, BN], F32, name="s2_ps")
                        for a_c in range(4):
                            nc.tensor.matmul(
                                e_ps[:], wfull[:, a_c:a_c + 1],
                                R[:, a_c * BN:(a_c + 1) * BN],
                                start=(a_c == 0), stop=(a_c == 3))
                        # hh part of gates -- PE-busy work under the softmax
                        for g_c in range(4):
                            for h_c in range(4):
                                nc.tensor.matmul(
                                    gps[g_c],
                                    h_store[:, h_c * 208 + hof: h_c * 208 + hof + 8],
                                    whh[:, h_c * G + g_c * 512: h_c * G + (g_c + 1) * 512],
                                    start=False, stop=False)
                        # softmax over n (no max-shift: e is O(4) at this
                        # model's scales so exp cannot overflow)
                        exps = sp.tile([1, BN], F32, name="exps", bufs=2)
                        nc.scalar.activation(exps[:], e_ps[:], AF.Exp)
                        sums = sp.tile([1, BL], F32, name="sums")
                        nc.vector.tensor_reduce(
                            sums[:],
                            exps[:].rearrange("p (b n) -> p b n", n=NP)[:, :, :N],
                            axis=AX.X, op=OP.add)
                        recip = sp.tile([1, BL], F32, name="recip")
                        nc.vector.reciprocal(recip[:], sums[:])
                        alpha = sp.tile([1, BN], F32, name="alpha", bufs=2)
                        nc.vector.tensor_tensor(
                            alpha[:].rearrange("p (b n) -> p b n", n=NP),
                            exps[:].rearrange("p (b n) -> p b n", n=NP),
                            recip[:].unsqueeze(2).broadcast_to([1, 8, NP]),
                            op=OP.mult)
                        nc.sync.dma_start(
                            out=alphas_d[:, s, :].unsqueeze(0),
                            in_=alpha[:].rearrange("p (b n) -> p b n", n=NP)[:, :, :N])
                        # transpose alpha -> [(b,n), 1] then scatter block-diag
                        aT = ps_pool.tile([128, 4], F32, name="s2_ps")
                        for bn_c in range(4):
                            nc.tensor.matmul(
                                aT[:, bn_c:bn_c + 1],
                                alpha[:, bn_c * 128:(bn_c + 1) * 128], id1[:],
                                start=True, stop=True)
                        # even b -> rows 0:64 cols {0,10,20,30}; odd -> rows 64:128 cols {1,11,21,31}
                        nc.vector.tensor_copy(alpha_diag[0:64, 0:31:10], aT[0:64, :])
                        nc.vector.tensor_copy(alpha_diag[64:128, 1:32:10], aT[64:128, :])
                        # wsum: the alpha-weighted sum of proj closes the gates
                        for g_c in range(4):
                            for bn_c in range(4):
                                nc.tensor.matmul(
                                    gps[g_c],
                                    alpha_diag[:, bn_c * 8:(bn_c + 1) * 8],
                                    proj[:, bn_c * G + g_c * 512: bn_c * G + (g_c + 1) * 512],
                                    start=False, stop=(bn_c == 3))
                        # LSTM pointwise (gate order i, f, g, o)
                        sig_i = sp.tile([8, 512], F32, name="sig_i")
                        nc.scalar.activation(sig_i[:], gps[0], AF.Sigmoid)
                        sig_f = sp.tile([8, 512], F32, name="sig_f")
                        nc.scalar.activation(sig_f[:], gps[1], AF.Sigmoid)
                        tanh_g = sp.tile([8, 512], F32, name="tanh_g")
                        nc.scalar.activation(tanh_g[:], gps[2], AF.Tanh)
                        sig_o = sp.tile([8, 512], F32, name="sig_o")
                        nc.scalar.activation(sig_o[:], gps[3], AF.Sigmoid)
                        c_prev, c_new = c_buf[s % 2], c_buf[(s + 1) % 2]
                        t1 = sp.tile([8, 512], F32, name="t1")
                        nc.vector.tensor_tensor(t1[:], sig_f[:], c_prev[:], op=OP.mult)
                        t2 = sp.tile([8, 512], F32, name="t2")
                        nc.vector.tensor_tensor(t2[:], sig_i[:], tanh_g[:], op=OP.mult)
                        nc.vector.tensor_tensor(c_new[:], t1[:], t2[:], op=OP.add)
                        tanh_c = sp.tile([8, 512], F32, name="tanh_c")
                        nc.scalar.activation(tanh_c[:], c_new[:], AF.Tanh)
                        h_new = sp.tile([8, 512], BF16, name="h_new", bufs=2)
                        nc.vector.tensor_tensor(h_new[:], sig_o[:], tanh_c[:], op=OP.mult)
                        # h_T -> h_store[s+1]
                        hT = ps_pool.tile([128, 32], F32, name="s1_ps")
                        for h_c in range(4):
                            nc.tensor.matmul(
                                hT[:, h_c * 8:(h_c + 1) * 8],
                                h_new[:, h_c * 128:(h_c + 1) * 128], id8[:],
                                start=True, stop=True)
                        nc.vector.tensor_copy(
                            h_store[:].rearrange("p (c t) -> p c t", c=4)[
                                :, :, hof + 8: hof + 16],
                            hT[:].rearrange("p (c b) -> p c b", b=8))
                        if s == 12:
                            emit_pred_half(0, 13)
                    emit_pred_half(13, 12)

    return nc


# ---------------------------------------------------------------- host prep
def _prep_core_inputs(features, captions, emb, W_ih, b_ih, W_hh, b_hh,
                      W_enc_att, b_enc_att, W_dec_att, b_dec_att, W_full, b_full,
                      W_init_h, b_init_h, W_init_c, b_init_c, W_fcn, b_fcn):
    f32 = np.float32
    embeds = np.asarray(emb, f32)[np.asarray(captions)[:, :T]]     # [B, T, E]
    shared = {
        "wdec": np.ascontiguousarray(np.asarray(W_dec_att, f32).T)   # [H, A]
                .reshape(4, 128, A).transpose(1, 0, 2).reshape(128, 4 * A).astype(BF),
        "wihe": np.ascontiguousarray(np.asarray(W_ih, f32)[:, :E].T)
                .reshape(4, 128, G).transpose(1, 0, 2).reshape(128, 4 * G).astype(BF),
        "wihc": np.ascontiguousarray(np.asarray(W_ih, f32)[:, E:].T)
                .reshape(16, 128, G).transpose(1, 0, 2).reshape(128, 16 * G).astype(BF),
        "whh": np.ascontiguousarray(np.asarray(W_hh, f32).T)
                .reshape(4, 128, G).transpose(1, 0, 2).reshape(128, 4 * G).astype(BF),
        "wenc": np.ascontiguousarray(np.asarray(W_enc_att, f32).T)
                .reshape(16, 128, A).transpose(1, 0, 2).reshape(128, 16 * A).astype(BF),
        "wfull": np.ascontiguousarray(np.asarray(W_full, f32).T)     # [A, 1]
                .reshape(4, 128).T.astype(BF),
        "wfcn": np.ascontiguousarray(np.asarray(W_fcn, f32).T)
                .reshape(4, 128, V).transpose(1, 0, 2).reshape(128, 4 * V).astype(BF),
        "winh": np.ascontiguousarray(np.asarray(W_init_h, f32).T)
                .reshape(16, 128, H).transpose(1, 0, 2).reshape(128, 16 * H).astype(BF),
        "winc": np.ascontiguousarray(np.asarray(W_init_c, f32).T)
                .reshape(16, 128, H).transpose(1, 0, 2).reshape(128, 16 * H).astype(BF),
        "batt": (np.asarray(b_enc_att, f32) + np.asarray(b_dec_att, f32))
                .reshape(1, A).astype(BF),
        "bg": (np.asarray(b_ih, f32) + np.asarray(b_hh, f32)).reshape(1, G).astype(BF),
        "binh": np.asarray(b_init_h, f32).reshape(1, H).astype(BF),
        "binc": np.asarray(b_init_c, f32).reshape(1, H).astype(BF),
        "bfcn": np.asarray(b_fcn, f32).reshape(1, V).astype(BF),
        "ones": np.ones((1, 128), BF),
        "onesbn": np.ones((1, BN), BF),
        "id8": np.eye(8, dtype=BF),
        "id1": np.ones((1, 1), f32),
        "maskbn": (np.arange(128).reshape(1, 128) % NP < N).astype(BF),
    }
    in_maps = []
    feats = np.asarray(features, f32)
    for k in range(NC_):
        bs = slice(k * BL, (k + 1) * BL)
        # featT[e, b*64+n] = features[b, n, e], zero pad n>=49
        ft = np.zeros((ENC, BL, NP), f32)
        ft[:, :, :N] = feats[bs].transpose(2, 0, 1)
        ft = ft.reshape(ENC, BN).reshape(16, 128, BN).transpose(1, 0, 2) \
               .reshape(128, 16 * BN).astype(BF)
        # embT[e, s*8+b]
        et = embeds[bs].transpose(2, 1, 0).reshape(E, T * BL) \
                       .reshape(4, 128, T * BL).transpose(1, 0, 2) \
                       .reshape(128, 4 * T * BL).astype(BF)
        in_maps.append({"featT": ft, "embT": et, **shared})
    return in_maps


def kernel(**inputs):
    global LAST_RESULTS
    _install_wait_split()
    from concourse.bass_utils import run_bass_kernel_spmd

    in_maps = _prep_core_inputs(**inputs)
    nc = build_program()
    res = run_bass_kernel_spmd(nc, in_maps, core_ids=list(range(NC_)),
                               trace=TRACE)
    LAST_RESULTS = res
    preds = np.concatenate([res.results[k]["preds"] for k in range(NC_)], axis=0)
    alphas = np.concatenate([res.results[k]["alphas"] for k in range(NC_)], axis=0)
    return preds, alphas


# revision 34
# speedup vs baseline: 1.0091x; 1.0091x over previous
"""DecoderRNN (show-attend-tell style) on 8 trn2 NeuronCores.

Strategy: data-parallel over batch (8 batches/core, weights replicated,
no collectives).  The per-step context matmul is folded into a one-time
precompute proj = features @ W_ih[:, E:].T so the per-step attention
weighted sum runs in gate space as a block-diagonal matmul on the PE.
The vocab projection runs after the scan as one batched matmul over all
(step, batch) hidden states.
"""
import numpy as np
import ml_dtypes

# ---- problem constants (hardcoded; kernel.py must be self-contained) ----
B, N, ENC = 64, 49, 2048
S = 26
V, E, H, A = 10000, 512, 512, 512
G = 4 * H            # 2048 gate dim
T = S - 1            # 25 decode steps
NC_ = 8              # cores
BL = B // NC_        # 8 batches per core
NP = 64              # n padded to 64 -> (b, n) pack = 512 rows
BN = BL * NP         # 512
VC = 500             # vocab chunk for phase 2
BF = ml_dtypes.bfloat16

TRACE = False        # test.py can flip this for profiled runs
LAST_RESULTS = None  # BassKernelResults stash for test.py


# ---------------------------------------------------------------- wait split
def _split_multi_waits_json(bir: dict) -> dict:
    """walrus in this env supports 1 sync-wait per instruction; split the
    extra semaphore waits Tile emits into standalone EventSemaphore insts."""
    n = 0
    for f in bir.get("functions", []):
        for bb in f.get("blocks", []):
            out = []
            for inst in bb.get("instructions", []):
                si = inst.get("sync_info")
                waits = (si or {}).get("on_wait") or []
                if len(waits) > 1:
                    sem = [w for w in waits if w.get("sync_type") == "semaphore"]
                    other = [w for w in waits if w.get("sync_type") != "semaphore"]
                    keep, move = (other, sem) if other else ([sem[-1]], sem[:-1])
                    for w in move:
                        n += 1
                        out.append({
                            "debug": inst.get("debug", 0),
                            "engine": inst["engine"],
                            "ins": [], "outs": [],
                            "name": f"{inst['name']}-ws{n}",
                            "opcode": "EventSemaphore",
                            "sync_info": {"on_update": [], "on_wait": [w]},
                        })
                    si["on_wait"] = keep
                out.append(inst)
            bb["instructions"] = out
    return bir


_PATCHED = False


def _install_wait_split():
    global _PATCHED
    if _PATCHED:
        return
    _PATCHED = True
    import orjson
    import concourse.bass as bass
    orig = bass.Bass.to_json_bytes

    def patched(self) -> bytes:
        return orjson.dumps(_split_multi_waits_json(orjson.loads(orig(self))))

    bass.Bass.to_json_bytes = patched


# ---------------------------------------------------------------- program
def build_program():
    import concourse.bass as bass
    import concourse.mybir as mybir
    from concourse.tile import TileContext

    BF16 = mybir.dt.bfloat16
    F32 = mybir.dt.float32
    AF = mybir.ActivationFunctionType
    OP = mybir.AluOpType
    AX = mybir.AxisListType

    nc = bass.Bass()
    dp = lambda name, shape, dt=BF16: nc.declare_dram_parameter(name, shape, dt, isOutput=False)

    # per-core inputs (already transposed/padded/bf16 on host)
    featT_d = dp("featT", [128, 16 * BN])          # [e_c, 128][bn]
    embT_d = dp("embT", [128, 4 * T * BL])         # [e_c, 128][s*8+b]
    wdec_d = dp("wdec", [128, 4 * A])              # lhsT[k=h][a], chunk h_c at a-col block
    wihe_d = dp("wihe", [128, 4 * G])              # rhs [k=e][g]
    wihc_d = dp("wihc", [128, 16 * G])             # rhs [k=enc][g]
    whh_d = dp("whh", [128, 4 * G])                # rhs [k=h][g]
    wenc_d = dp("wenc", [128, 16 * A])             # lhsT [k=enc][a]
    wfull_d = dp("wfull", [128, 4])                # lhsT [k=a][1] per chunk col
    wfcn_d = dp("wfcn", [128, 4 * V])              # rhs [k=h][v]
    winh_d = dp("winh", [128, 16 * H])             # rhs [k=enc][h]
    winc_d = dp("winc", [128, 16 * H])             # rhs [k=enc][h]
    batt_d = dp("batt", [1, A])                    # b_enc_att + b_dec_att
    bg_d = dp("bg", [1, G])                        # b_ih + b_hh
    binh_d = dp("binh", [1, H])
    binc_d = dp("binc", [1, H])
    bfcn_d = dp("bfcn", [1, V])
    ones_d = dp("ones", [1, 128])
    onesbn_d = dp("onesbn", [1, BN])
    id8_d = dp("id8", [8, 8])
    id1_d = dp("id1", [1, 1], F32)
    maskbn_d = dp("maskbn", [1, 128])

    preds_d = nc.declare_dram_parameter("preds", [BL, T, V], F32, isOutput=True)
    alphas_d = nc.declare_dram_parameter("alphas", [BL, T, N], F32, isOutput=True)

    with TileContext(nc) as tc:
        with tc.tile_pool(name="main", bufs=1) as mp:
            # ---- persistent SBUF ----
            # tiles for scan weights (DMAs issued after precompute-critical
            # loads so they don't delay featT/wihc on the DMA queues)
            wihe = mp.tile([128, 4 * G], BF16)
            whh = mp.tile([128, 4 * G], BF16)
            wdec = mp.tile([128, 4 * A], BF16)
            wfull = mp.tile([128, 4], BF16)
            embT = mp.tile([128, 4 * T * BL], BF16)
            bg = mp.tile([1, G], BF16)
            nc.sync.dma_start(out=bg[:], in_=bg_d[:])
            bfcn = mp.tile([1, V], BF16)
            ones = mp.tile([1, 128], BF16)
            nc.sync.dma_start(out=ones[:], in_=ones_d[:])
            id8 = mp.tile([8, 8], BF16)
            id1 = mp.tile([1, 1], F32)
            maskbn = mp.tile([1, 128], BF16)
            nc.sync.dma_start(out=maskbn[:], in_=maskbn_d[:])

            proj = mp.tile([128, 4 * G], BF16)        # [(b,n) chunk][g]
            att1 = mp.tile([128, 4 * BN], BF16)       # [a chunk][bn]
            h_store = mp.tile([128, S * 32], BF16)    # col h_c*208 + s*8 + b
            alpha_diag = mp.tile([128, 32], BF16)     # col bn_c*8 + b
            nc.vector.memset(alpha_diag[:], 0.0)
            c_buf = [mp.tile([8, H], F32, name=f"cbuf{i}") for i in range(2)]

            # =================== PRECOMPUTE ===================
            with (
                tc.tile_pool(name="pre", bufs=1) as pp,
                tc.tile_pool(name="pre_dma", bufs=2) as pd,
                tc.tile_pool(name="pre_ps", bufs=1, space="PSUM") as pps,
            ):
                featT = pp.tile([128, 16 * BN], BF16)
                for c in range(8):
                    w = 2 * BN
                    nc.sync.dma_start(out=featT[:, c * w:(c + 1) * w],
                                      in_=featT_d[:, c * w:(c + 1) * w])
                wenc = pp.tile([128, 16 * A], BF16)
                for c in range(4):
                    w = 4 * A
                    nc.sync.dma_start(out=wenc[:, c * w:(c + 1) * w],
                                      in_=wenc_d[:, c * w:(c + 1) * w])
                batt = pp.tile([1, A], BF16)
                nc.sync.dma_start(out=batt[:], in_=batt_d[:])
                binh = pp.tile([1, H], BF16)
                nc.sync.dma_start(out=binh[:], in_=binh_d[:])
                binc = pp.tile([1, H], BF16)
                nc.sync.dma_start(out=binc[:], in_=binc_d[:])
                onesbn = pp.tile([1, BN], BF16)
                nc.sync.dma_start(out=onesbn[:], in_=onesbn_d[:])
                winh_s = pp.tile([128, 16 * H], BF16)
                winc_s = pp.tile([128, 16 * H], BF16)
                for c in range(4):
                    w = 4 * H
                    nc.sync.dma_start(out=winh_s[:, c * w:(c + 1) * w],
                                      in_=winh_d[:, c * w:(c + 1) * w])
                    nc.sync.dma_start(out=winc_s[:, c * w:(c + 1) * w],
                                      in_=winc_d[:, c * w:(c + 1) * w])

                # mean_enc (pad cols are zero so plain sum/49 is exact)
                meanf = pp.tile([128, 16 * BL], F32)
                for e_c in range(16):
                    nc.vector.tensor_reduce(
                        meanf[:, e_c * BL:(e_c + 1) * BL],
                        featT[:, e_c * BN:(e_c + 1) * BN].rearrange(
                            "p (b n) -> p b n", n=NP),
                        axis=AX.X, op=OP.add)
                mean = pp.tile([128, 16 * BL], BF16)
                nc.vector.tensor_scalar_mul(mean[:], meanf[:], 1.0 / N)

                # proj = features @ W_ihc.T  -> [(b,n), g]
                for g_c in range(4):
                    wslab = pd.tile([128, 16 * 512], BF16, name="wslab")
                    nc.sync.dma_start(
                        out=wslab[:],
                        in_=wihc_d[:].rearrange("p (e g) -> p e g", g=G)[
                            :, :, g_c * 512:(g_c + 1) * 512])
                    for bn_c in range(4):
                        ps = pps.tile([128, 512], F32, name="proj_ps", bufs=2)
                        for e_c in range(16):
                            nc.tensor.matmul(
                                ps[:],
                                featT[:, e_c * BN + bn_c * 128: e_c * BN + (bn_c + 1) * 128],
                                wslab[:, e_c * 512:(e_c + 1) * 512],
                                start=(e_c == 0), stop=False)
                        nc.tensor.matmul(
                            ps[:], maskbn[:], bg[:, g_c * 512:(g_c + 1) * 512],
                            start=False, stop=True)
                        nc.vector.tensor_copy(
                            proj[:, bn_c * G + g_c * 512: bn_c * G + (g_c + 1) * 512],
                            ps[:])

                # att1 = features @ W_enc_att.T + (b_enc+b_dec)  -> [a, (b,n)]
                for a_c in range(4):
                    ps = pps.tile([128, BN], F32, name="att1_ps")
                    for e_c in range(16):
                        nc.tensor.matmul(
                            ps[:],
                            wenc[:, e_c * A + a_c * 128: e_c * A + (a_c + 1) * 128],
                            featT[:, e_c * BN:(e_c + 1) * BN],
                            start=(e_c == 0), stop=False)
                    nc.tensor.matmul(
                        ps[:], batt[:, a_c * 128:(a_c + 1) * 128], onesbn[:],
                        start=False, stop=True)
                    nc.vector.tensor_copy(
                        att1[:, a_c * BN:(a_c + 1) * BN], ps[:])

                # scan weights can stream in while proj/att1 matmuls run
                nc.sync.dma_start(out=wdec[:], in_=wdec_d[:])
                nc.sync.dma_start(out=wfull[:], in_=wfull_d[:])
                nc.sync.dma_start(out=wihe[:], in_=wihe_d[:])
                nc.sync.dma_start(out=whh[:], in_=whh_d[:])
                nc.sync.dma_start(out=embT[:], in_=embT_d[:])
                nc.sync.dma_start(out=id8[:], in_=id8_d[:])
                nc.sync.dma_start(out=id1[:], in_=id1_d[:])
                nc.sync.dma_start(out=bfcn[:], in_=bfcn_d[:])

                # h0 / c0
                h0ps = pps.tile([8, H], F32, name="h0_ps")
                c0ps = pps.tile([8, H], F32, name="c0_ps")
                for e_c in range(16):
                    nc.tensor.matmul(h0ps[:], mean[:, e_c * BL:(e_c + 1) * BL],
                                     winh_s[:, e_c * H:(e_c + 1) * H],
                                     start=(e_c == 0), stop=False)
                    nc.tensor.matmul(c0ps[:], mean[:, e_c * BL:(e_c + 1) * BL],
                                     winc_s[:, e_c * H:(e_c + 1) * H],
                                     start=(e_c == 0), stop=False)
                nc.tensor.matmul(h0ps[:], ones[:, :BL], binh[:], start=False, stop=True)
                nc.tensor.matmul(c0ps[:], ones[:, :BL], binc[:], start=False, stop=True)
                nc.vector.tensor_copy(c_buf[0][:], c0ps[:])
                h0 = pp.tile([8, H], BF16)
                nc.vector.tensor_copy(h0[:], h0ps[:])
                hTps = pps.tile([128, 32], F32, name="hT_ps0")
                for h_c in range(4):
                    nc.tensor.matmul(hTps[:, h_c * 8:(h_c + 1) * 8],
                                     h0[:, h_c * 128:(h_c + 1) * 128], id8[:],
                                     start=True, stop=True)
                for h_c in range(4):
                    nc.vector.tensor_copy(
                        h_store[:, h_c * 208: h_c * 208 + 8],
                        hTps[:, h_c * 8:(h_c + 1) * 8])

            # =================== SCAN + VOCAB ===================
            with (
                tc.tile_pool(name="fcnp", bufs=1) as fp,
                tc.tile_pool(name="step", bufs=1) as sp,
            ):
                wfcn = fp.tile([128, 4 * V], BF16)
                for c in range(8):
                    w = V // 2
                    nc.sync.dma_start(out=wfcn[:, c * w:(c + 1) * w],
                                      in_=wfcn_d[:, c * w:(c + 1) * w])

                with tc.tile_pool(name="scan_ps", bufs=1, space="PSUM") as ps_pool:
                    def emit_pred_half(s0, ns):
                        # preds[b, s, :] = h_{s+1} @ W_fcn.T + b_fcn
                        m = ns * 8
                        for v_c in range(V // VC):
                            ps = ps_pool.tile([104, VC], F32, name="pred_ps",
                                              bufs=2)
                            for h_c in range(4):
                                nc.tensor.matmul(
                                    ps[:m],
                                    h_store[:, h_c * 208 + (1 + s0) * 8:
                                            h_c * 208 + (1 + s0) * 8 + m],
                                    wfcn[:, h_c * V + v_c * VC: h_c * V + (v_c + 1) * VC],
                                    start=(h_c == 0), stop=False)
                            nc.tensor.matmul(
                                ps[:m], ones[:, :m], bfcn[:, v_c * VC:(v_c + 1) * VC],
                                start=False, stop=True)
                            pout = sp.tile([104, VC], F32, name="pout", bufs=3)
                            nc.vector.tensor_copy(pout[:m], ps[:m])
                            nc.sync.dma_start(
                                out=preds_d[:].rearrange("b s v -> s b v")[
                                    s0:s0 + ns, :, v_c * VC:(v_c + 1) * VC],
                                in_=pout[:m])

                    for s in range(T):
                        hof = s * 8
                        # att2_T[a, b] = W_dec_att @ h
                        att2 = ps_pool.tile([128, 32], F32, name="s1_ps")
                        for a_c in range(4):
                            for h_c in range(4):
                                nc.tensor.matmul(
                                    att2[:, a_c * 8:(a_c + 1) * 8],
                                    wdec[:, h_c * A + a_c * 128: h_c * A + (a_c + 1) * 128],
                                    h_store[:, h_c * 208 + hof: h_c * 208 + hof + 8],
                                    start=(h_c == 0), stop=(h_c == 3))
                        # R = relu(att1 + att2 bcast over n)  (bf16, PE rhs)
                        R = sp.tile([128, 4 * BN], BF16, name="R", bufs=2)
                        nc.vector.tensor_tensor(
                            R[:].rearrange("p (c b n) -> p c b n", b=8, n=NP),
                            att1[:].rearrange("p (c b n) -> p c b n", b=8, n=NP),
                            att2[:].rearrange("p (c b) -> p c b", b=8)
                                .unsqueeze(3).broadcast_to([128, 4, 8, NP]),
                            op=OP.add)
                        nc.vector.tensor_scalar_max(R[:], R[:], 0.0)
                        # gates accumulation starts with the emb part -- emitted
                        # here so the PE chews on it while DVE runs relu/softmax
                        gps = [ps_pool.tile([8, 512], F32, name=f"g{g_c}_ps")[:]
                               for g_c in range(4)]
                        for g_c in range(4):
                            for e_c in range(4):
                                nc.tensor.matmul(
                                    gps[g_c],
                                    embT[:, e_c * T * BL + s * BL: e_c * T * BL + (s + 1) * BL],
                                    wihe[:, e_c * G + g_c * 512: e_c * G + (g_c + 1) * 512],
                                    start=(e_c == 0), stop=False)
                        # e[(b,n)] = W_full . R
                        e_ps = ps_pool.tile([1, BN], F32, name="s2_ps")
                        for a_c in range(4):
                            nc.tensor.matmul(
                                e_ps[:], wfull[:, a_c:a_c + 1],
                                R[:, a_c * BN:(a_c + 1) * BN],
                                start=(a_c == 0), stop=(a_c == 3))
                        # hh part of gates -- PE-busy work under the softmax
                        for g_c in range(4):
                            for h_c in range(4):
                                nc.tensor.matmul(
                                    gps[g_c],
                                    h_store[:, h_c * 208 + hof: h_c * 208 + hof + 8],
                                    whh[:, h_c * G + g_c * 512: h_c * G + (g_c + 1) * 512],
                                    start=False, stop=False)
                        # softmax over n (no max-shift: e is O(4) at this
                        # model's scales so exp cannot overflow)
                        exps = sp.tile([1, BN], F32, name="exps", bufs=2)
                        nc.scalar.activation(exps[:], e_ps[:], AF.Exp)
                        sums = sp.tile([1, BL], F32, name="sums")
                        nc.vector.tensor_reduce(
                            sums[:],
                            exps[:].rearrange("p (b n) -> p b n", n=NP)[:, :, :N],
                            axis=AX.X, op=OP.add)
                        recip = sp.tile([1, BL], F32, name="recip")
                        nc.vector.reciprocal(recip[:], sums[:])
                        alpha = sp.tile([1, BN], F32, name="alpha", bufs=2)
                        nc.vector.tensor_tensor(
                            alpha[:].rearrange("p (b n) -> p b n", n=NP),
                            exps[:].rearrange("p (b n) -> p b n", n=NP),
                            recip[:].unsqueeze(2).broadcast_to([1, 8, NP]),
                            op=OP.mult)
                        nc.sync.dma_start(
                            out=alphas_d[:, s, :].unsqueeze(0),
                            in_=alpha[:].rearrange("p (b n) -> p b n", n=NP)[:, :, :N])
                        # transpose alpha -> [(b,n), 1] then scatter block-diag
                        aT = ps_pool.tile([128, 4], F32, name="s2_ps")
                        for bn_c in range(4):
                            nc.tensor.matmul(
                                aT[:, bn_c:bn_c + 1],
                                alpha[:, bn_c * 128:(bn_c + 1) * 128], id1[:],
                                start=True, stop=True)
                        # even b -> rows 0:64 cols {0,10,20,30}; odd -> rows 64:128 cols {1,11,21,31}
                        nc.vector.tensor_copy(alpha_diag[0:64, 0:31:10], aT[0:64, :])
                        nc.vector.tensor_copy(alpha_diag[64:128, 1:32:10], aT[64:128, :])
                        # wsum: the alpha-weighted sum of proj closes the gates
                        for g_c in range(4):
                            for bn_c in range(4):
                                nc.tensor.matmul(
                                    gps[g_c],
                                    alpha_diag[:, bn_c * 8:(bn_c + 1) * 8],
                                    proj[:, bn_c * G + g_c * 512: bn_c * G + (g_c + 1) * 512],
                                    start=False, stop=(bn_c == 3))
                        # LSTM pointwise (gate order i, f, g, o)
                        sig_i = sp.tile([8, 512], F32, name="sig_i")
                        nc.scalar.activation(sig_i[:], gps[0], AF.Sigmoid)
                        sig_f = sp.tile([8, 512], F32, name="sig_f")
                        nc.scalar.activation(sig_f[:], gps[1], AF.Sigmoid)
                        tanh_g = sp.tile([8, 512], F32, name="tanh_g")
                        nc.scalar.activation(tanh_g[:], gps[2], AF.Tanh)
                        sig_o = sp.tile([8, 512], F32, name="sig_o")
                        nc.scalar.activation(sig_o[:], gps[3], AF.Sigmoid)
                        c_prev, c_new = c_buf[s % 2], c_buf[(s + 1) % 2]
                        t1 = sp.tile([8, 512], F32, name="t1")
                        nc.vector.tensor_tensor(t1[:], sig_f[:], c_prev[:], op=OP.mult)
                        t2 = sp.tile([8, 512], F32, name="t2")
                        nc.vector.tensor_tensor(t2[:], sig_i[:], tanh_g[:], op=OP.mult)
                        nc.vector.tensor_tensor(c_new[:], t1[:], t2[:], op=OP.add)
                        tanh_c = sp.tile([8, 512], F32, name="tanh_c")
                        nc.scalar.activation(tanh_c[:], c_new[:], AF.Tanh)
                        h_new = sp.tile([8, 512], BF16, name="h_new", bufs=2)
                        nc.vector.tensor_tensor(h_new[:], sig_o[:], tanh_c[:], op=OP.mult)
                        # h_T -> h_store[s+1]
                        hT = ps_pool.tile([128, 32], F32, name="s1_ps")
                        for h_c in range(4):
                            nc.tensor.matmul(
                                hT[:, h_c * 8:(h_c + 1) * 8],
                                h_new[:, h_c * 128:(h_c + 1) * 128], id8[:],
                                start=True, stop=True)
                        nc.vector.tensor_copy(
                            h_store[:].rearrange("p (c t) -> p c t", c=4)[
                                :, :, hof + 8: hof + 16],
                            hT[:].rearrange("p (c b) -> p c b", b=8))
                        if s == 12:
                            emit_pred_half(0, 13)
                    emit_pred_half(13, 12)

    return nc


# ---------------------------------------------------------------- host prep
def _prep_core_inputs(features, captions, emb, W_ih, b_ih, W_hh, b_hh,
                      W_enc_att, b_enc_att, W_dec_att, b_dec_att, W_full, b_full,
                      W_init_h, b_init_h, W_init_c, b_init_c, W_fcn, b_fcn):
    f32 = np.float32
    embeds = np.asarray(emb, f32)[np.asarray(captions)[:, :T]]     # [B, T, E]
    shared = {
        "wdec": np.ascontiguousarray(np.asarray(W_dec_att, f32).T)   # [H, A]
                .reshape(4, 128, A).transpose(1, 0, 2).reshape(128, 4 * A).astype(BF),
        "wihe": np.ascontiguousarray(np.asarray(W_ih, f32)[:, :E].T)
                .reshape(4, 128, G).transpose(1, 0, 2).reshape(128, 4 * G).astype(BF),
        "wihc": np.ascontiguousarray(np.asarray(W_ih, f32)[:, E:].T)
                .reshape(16, 128, G).transpose(1, 0, 2).reshape(128, 16 * G).astype(BF),
        "whh": np.ascontiguousarray(np.asarray(W_hh, f32).T)
                .reshape(4, 128, G).transpose(1, 0, 2).reshape(128, 4 * G).astype(BF),
        "wenc": np.ascontiguousarray(np.asarray(W_enc_att, f32).T)
                .reshape(16, 128, A).transpose(1, 0, 2).reshape(128, 16 * A).astype(BF),
        "wfull": np.ascontiguousarray(np.asarray(W_full, f32).T)     # [A, 1]
                .reshape(4, 128).T.astype(BF),
        "wfcn": np.ascontiguousarray(np.asarray(W_fcn, f32).T)
                .reshape(4, 128, V).transpose(1, 0, 2).reshape(128, 4 * V).astype(BF),
        "winh": np.ascontiguousarray(np.asarray(W_init_h, f32).T)
                .reshape(16, 128, H).transpose(1, 0, 2).reshape(128, 16 * H).astype(BF),
        "winc": np.ascontiguousarray(np.asarray(W_init_c, f32).T)
                .reshape(16, 128, H).transpose(1, 0, 2).reshape(128, 16 * H).astype(BF),
        "batt": (np.asarray(b_enc_att, f32) + np.asarray(b_dec_att, f32))
                .reshape(1, A).astype(BF),
        "bg": (np.asarray(b_ih, f32) + np.asarray(b_hh, f32)).reshape(1, G).astype(BF),
        "binh": np.asarray(b_init_h, f32).reshape(1, H).astype(BF),
        "binc": np.asarray(b_init_c, f32).reshape(1, H).astype(BF),
        "bfcn": np.asarray(b_fcn, f32).reshape(1, V).astype(BF),
        "ones": np.ones((1, 128), BF),
        "onesbn": np.ones((1, BN), BF),
        "id8": np.eye(8, dtype=BF),
        "id1": np.ones((1, 1), f32),
        "maskbn": (np.arange(128).reshape(1, 128) % NP < N).astype(BF),
    }
    in_maps = []
    feats = np.asarray(features, f32)
    for k in range(NC_):
        bs = slice(k * BL, (k + 1) * BL)
        # featT[e, b*64+n] = features[b, n, e], zero pad n>=49
        ft = np.zeros((ENC, BL, NP), f32)
        ft[:, :, :N] = feats[bs].transpose(2, 0, 1)
        ft = ft.reshape(ENC, BN).reshape(16, 128, BN).transpose(1, 0, 2) \
               .reshape(128, 16 * BN).astype(BF)
        # embT[e, s*8+b]
        et = embeds[bs].transpose(2, 1, 0).reshape(E, T * BL) \
                       .reshape(4, 128, T * BL).transpose(1, 0, 2) \
                       .reshape(128, 4 * T * BL).astype(BF)
        in_maps.append({"featT": ft, "embT": et, **shared})
    return in_maps


def kernel(**inputs):
    global LAST_RESULTS
    _install_wait_split()
    from concourse.bass_utils import run_bass_kernel_spmd

    in_maps = _prep_core_inputs(**inputs)
    nc = build_program()
    res = run_bass_kernel_spmd(nc, in_maps, core_ids=list(range(NC_)),
                               trace=TRACE)
    LAST_RESULTS = res
    preds = np.concatenate([res.results[k]["preds"] for k in range(NC_)], axis=0)
    alphas = np.concatenate([res.results[k]["alphas"] for k in range(NC_)], axis=0)
    return preds, alphas
